# revision 38
# baseline (speedup 1.0000x reference)
"""Additive attention scorer: S[b,q,k] = sum_h wv[h] * tanh((qs@Wq)[b,q,h] + (ks@Wk)[b,k,h]).

Sharding: data-parallel over batch B=8 across the 8 NeuronCores (one batch
element per core). Per core:
  1. PE projects qT = Wq^T @ qs[b]^T and kT = Wk^T @ ks[b]^T  -> [H=128, 512]
     (H lands exactly on the 128 partitions). Inputs come in bf16 (the
     tanh-argument error budget tolerates it; halves DMA and keeps the
     matmuls single-pass -- fp32 matmuls decompose into 2 HW passes).
  2. For each query row lq: DVE builds X = kT + qT[:, lq] (per-partition
     scalar broadcast add, fp32 2x_2P mode), batched ~16 rows per tile.
  3. ACT applies one big tanh over the batched tile, fp32-in/bf16-out
     (amortizes the per-instruction overhead; fp32 reads run at ~1.02
     cyc/elem vs 1.25 for bf16 reads). ACT is the hard bottleneck:
     512*512*128 tanh / 128 lanes / 1.2GHz ~= 218us/core.
  4. PE reduces over H with a shifted-wv stationary trick: lhsT is a
     [128,128] window of a [128,255] bf16 tensor holding wv in column 127,
     so the single nonzero output row of each matmul lands on PSUM partition
     (lq mod 128); 128 matmuls accumulate one [128,512] fp32 output block.
  5. DVE copies PSUM->SBUF (bf16), one 4KB-descriptor DMA to DRAM at the end.
"""

import numpy as np

B, LQ, LK, D, H = 8, 512, 512, 512, 128
P = 128           # SBUF partitions
ND = D // P       # contraction chunks for the projections

_cache = {}


def _build():
    import concourse.bass as bass
    import concourse.tile as tile
    from concourse import bacc, mybir

    f32 = mybir.dt.float32
    bf16 = mybir.dt.bfloat16
    nc = bacc.Bacc("TRN2", target_bir_lowering=False, debug=False, num_devices=B)

    qsT = nc.dram_tensor("qsT", [D, LQ], bf16, kind="ExternalInput")
    ksT = nc.dram_tensor("ksT", [D, LK], bf16, kind="ExternalInput")
    Wq = nc.dram_tensor("Wq", [D, H], bf16, kind="ExternalInput")
    Wk = nc.dram_tensor("Wk", [D, H], bf16, kind="ExternalInput")
    wbig = nc.dram_tensor("wbig", [H, 2 * H - 1], bf16, kind="ExternalInput")
    out = nc.dram_tensor("out", [LQ, LK], bf16, kind="ExternalOutput")

    with tile.TileContext(nc) as tc:
        with (
            tc.tile_pool(name="const", bufs=1) as cpool,
            tc.tile_pool(name="load", bufs=1) as lpool,
            tc.tile_pool(name="x", bufs=4) as xpool,
            tc.tile_pool(name="y", bufs=2) as ypool,
            tc.tile_pool(name="ppsum", bufs=2, space=bass.MemorySpace.PSUM) as ppool,
            tc.tile_pool(name="spsum", bufs=2, space=bass.MemorySpace.PSUM) as sppool,
        ):
            # Pre-warm the ACT tanh table set while the input DMAs are in
            # flight (the PSEUDO_LOAD_ACT_FUNC_SET costs ~2.7us once).
            warm = cpool.tile([P, 1], f32, tag="warm")
            nc.vector.memset(warm[:], 0.0)
            nc.scalar.activation(
                warm[:], warm[:], mybir.ActivationFunctionType.Tanh
            )
            # All output blocks accumulate here; a single 4KB-descriptor DMA
            # writes them out at the end (host unpermutes rows).
            s_acc = cpool.tile([P, ND, LK], bf16, tag="sacc")

            # Projections: dst = W^T @ srcT accumulated over ND chunks of D.
            # One big DMA per input tensor -- a single InstDMACopy is split
            # across all 16 SDMA engines of its queue, so this parallelizes
            # better than per-chunk DMAs (which serialize on SWDGE issue).
            qT = cpool.tile([H, LQ], f32, tag="qT")
            kT = cpool.tile([H, LK], f32, tag="kT")
            srcs = {}
            # Weight tensors lead their queue (FIFO per queue): wk before ks
            # on sync, wq before qs on gpsimd -- each projection's operands
            # arrive in dependency order on its own queue.
            for name, dram, eng in (("wk", Wk, nc.sync), ("wq", Wq, nc.gpsimd)):
                t = lpool.tile([P, ND, H], bf16, tag=name)
                eng.dma_start(
                    t[:], dram[:, :].rearrange("(p c) k -> p c k", p=P)
                )
                srcs[name] = t
            # "(p c)" split: partition p holds DRAM rows 4p..4p+3, i.e. 4KB
            # contiguous per partition -> 4x bigger DMA descriptors. The
            # projection contracts over all of D either way, so the permuted
            # d-to-partition mapping is harmless (both operands use the same
            # mapping). Both ride the sync HWDGE queue: each InstDMACopy is
            # split across all 16 SDMA engines, and the scalar queue must be
            # left alone (it starves once ACT saturates).
            for name, dram, eng in (("ks", ksT, nc.sync), ("qs", qsT, nc.gpsimd)):
                t = lpool.tile([P, ND, LK], bf16, tag=name)
                src_r = dram[:, :].rearrange("(p c) k -> p c k", p=P)
                # Two halves so the first projection matmuls can start while
                # the second half streams; ks and qs ride different queues
                # (sync HWDGE / gpsimd SWDGE) so they stream in parallel.
                eng.dma_start(t[:, : ND // 2, :], src_r[:, : ND // 2, :])
                eng.dma_start(t[:, ND // 2 :, :], src_r[:, ND // 2 :, :])
                srcs[name] = t
            # wbig is not needed until the first reduce-matmul (~22us), so it
            # goes last on the gpsimd queue.
            wb = cpool.tile([P, 2 * H - 1], bf16, tag="wb")
            nc.gpsimd.dma_start(wb[:], wbig[:])
            for sname, wname, dst in (("ks", "wk", kT), ("qs", "wq", qT)):
                ps = ppool.tile([H, LQ], f32, tag="proj")
                for c in range(ND):
                    nc.tensor.matmul(
                        ps[:],
                        srcs[wname][:, c, :],
                        srcs[sname][:, c, :],
                        start=(c == 0),
                        stop=(c == ND - 1),
                    )
                nc.vector.tensor_copy(dst[:], ps[:])

            tanh = mybir.ActivationFunctionType.Tanh
            # Variable group sizes: small groups at the start (pipeline ramp:
            # ACT can begin after only 8 DVE adds) and a split last group so
            # the final reduce-matmuls overlap the tanh tail.
            sizes = [8] * 6 + [16] * 28 + [8] * 2
            assert sum(sizes) == LQ
            sp = None
            lq0 = 0
            for gidx, n in enumerate(sizes):
                if lq0 % P == 0:
                    sp = sppool.tile([P, LK], f32, tag="spsum")
                y = ypool.tile([P, n * LK], bf16, tag="y")
                x = xpool.tile([P, n * LK], f32, tag="x")
                for j in range(n):
                    lq = lq0 + j
                    nc.vector.tensor_scalar_add(
                        x[:, j * LK : (j + 1) * LK], kT[:], qT[:, lq : lq + 1]
                    )
                if gidx == 0 or gidx == len(sizes) - 1:
                    h = n // 2 * LK
                    nc.scalar.activation(y[:, :h], x[:, :h], tanh)
                    nc.scalar.activation(y[:, h:], x[:, h:], tanh)
                else:
                    nc.scalar.activation(y[:], x[:], tanh)
                for j in range(n):
                    r = (lq0 + j) % P
                    nc.tensor.matmul(
                        sp[:],
                        wb[:, H - 1 - r : 2 * H - 1 - r],
                        y[:, j * LK : (j + 1) * LK],
                        start=(r == 0),
                        stop=(r == P - 1),
                    )
                lq0 += n
                if lq0 % P == 0:
                    nc.vector.tensor_copy(s_acc[:, lq0 // P - 1, :], sp[:])
            # One DMA for the whole output: partition p holds S rows
            # {blk*128+p}, written to DRAM rows 4p+blk (4KB contiguous per
            # partition); the host transposes them back.
            nc.sync.dma_start(
                out[:, :].rearrange("(p c) k -> p c k", p=P), s_acc[:]
            )

    nc.compile()
    return nc


def _in_maps(qs, ks, Wq, Wk, wv):
    import ml_dtypes

    bf = ml_dtypes.bfloat16
    wbig = np.zeros((H, 2 * H - 1), np.float32)
    wbig[:, H - 1] = wv
    wbig = wbig.astype(bf)
    Wq_b = np.ascontiguousarray(Wq, dtype=np.float32).astype(bf)
    Wk_b = np.ascontiguousarray(Wk, dtype=np.float32).astype(bf)
    qs = np.asarray(qs)
    ks = np.asarray(ks)
    maps = []
    for b in range(B):
        maps.append(
            {
                "qsT": np.ascontiguousarray(qs[b].T).astype(bf),
                "ksT": np.ascontiguousarray(ks[b].T).astype(bf),
                "Wq": Wq_b,
                "Wk": Wk_b,
                "wbig": wbig,
            }
        )
    return maps


def _unpermute(o):
    # DRAM row p*4+blk holds S row blk*128+p (see the final out-DMA).
    o = np.asarray(o, dtype=np.float32)
    return o.reshape(P, ND, LK).transpose(1, 0, 2).reshape(LQ, LK)


def run(qs, ks, Wq, Wk, wv, trace=False):
    import time

    from concourse.bass_utils import run_bass_kernel_spmd

    if "nc" not in _cache:
        _cache["nc"] = _build()
    in_maps = _in_maps(qs, ks, Wq, Wk, wv)
    last_err = None
    for attempt in range(3):
        try:
            res = run_bass_kernel_spmd(
                _cache["nc"], in_maps, core_ids=list(range(B)), trace=trace
            )
            break
        except Exception as e:  # transient NRT_EXEC_UNIT_UNRECOVERABLE etc.
            last_err = e
            time.sleep(5 * (attempt + 1))
    else:
        raise last_err
    outs = np.stack([_unpermute(res.results[i]["out"]) for i in range(B)], axis=0)
    return outs, res


def kernel(qs, ks, Wq, Wk, wv):
    out, _ = run(qs, ks, Wq, Wk, wv, trace=False)
    return out


# revision 40
# speedup vs baseline: 1.0051x; 1.0051x over previous
"""Additive attention scorer: S[b,q,k] = sum_h wv[h] * tanh((qs@Wq)[b,q,h] + (ks@Wk)[b,k,h]).

Sharding: data-parallel over batch B=8 across the 8 NeuronCores (one batch
element per core). Per core:
  1. PE projects qT = Wq^T @ qs[b]^T and kT = Wk^T @ ks[b]^T  -> [H=128, 512]
     (H lands exactly on the 128 partitions). Inputs come in bf16 (the
     tanh-argument error budget tolerates it; halves DMA and keeps the
     matmuls single-pass -- fp32 matmuls decompose into 2 HW passes).
  2. For each query row lq: DVE builds X = kT + qT[:, lq] (per-partition
     scalar broadcast add, fp32 2x_2P mode), batched ~16 rows per tile.
  3. ACT applies one big tanh over the batched tile, fp32-in/bf16-out
     (amortizes the per-instruction overhead; fp32 reads run at ~1.02
     cyc/elem vs 1.25 for bf16 reads). ACT is the hard bottleneck:
     512*512*128 tanh / 128 lanes / 1.2GHz ~= 218us/core.
  4. PE reduces over H with a shifted-wv stationary trick: lhsT is a
     [128,128] window of a [128,255] bf16 tensor holding wv in column 127,
     so the single nonzero output row of each matmul lands on PSUM partition
     (lq mod 128); 128 matmuls accumulate one [128,512] fp32 output block.
  5. DVE copies PSUM->SBUF (bf16), one 4KB-descriptor DMA to DRAM at the end.
"""

import numpy as np

B, LQ, LK, D, H = 8, 512, 512, 512, 128
P = 128           # SBUF partitions
ND = D // P       # contraction chunks for the projections

_cache = {}


def _build():
    import concourse.bass as bass
    import concourse.tile as tile
    from concourse import bacc, mybir

    f32 = mybir.dt.float32
    bf16 = mybir.dt.bfloat16
    nc = bacc.Bacc("TRN2", target_bir_lowering=False, debug=False, num_devices=B)

    qsT = nc.dram_tensor("qsT", [D, LQ], bf16, kind="ExternalInput")
    ksT = nc.dram_tensor("ksT", [D, LK], bf16, kind="ExternalInput")
    Wq = nc.dram_tensor("Wq", [D, H], bf16, kind="ExternalInput")
    Wk = nc.dram_tensor("Wk", [D, H], bf16, kind="ExternalInput")
    wbig = nc.dram_tensor("wbig", [H, 2 * H - 1], bf16, kind="ExternalInput")
    out = nc.dram_tensor("out", [LQ, LK], bf16, kind="ExternalOutput")

    with tile.TileContext(nc) as tc:
        with (
            tc.tile_pool(name="const", bufs=1) as cpool,
            tc.tile_pool(name="load", bufs=1) as lpool,
            tc.tile_pool(name="x", bufs=4) as xpool,
            tc.tile_pool(name="y", bufs=2) as ypool,
            tc.tile_pool(name="ppsum", bufs=2, space=bass.MemorySpace.PSUM) as ppool,
            tc.tile_pool(name="spsum", bufs=2, space=bass.MemorySpace.PSUM) as sppool,
        ):
            # Pre-warm the ACT tanh table set while the input DMAs are in
            # flight (the PSEUDO_LOAD_ACT_FUNC_SET costs ~2.7us once).
            warm = cpool.tile([P, 1], f32, tag="warm")
            nc.vector.memset(warm[:], 0.0)
            nc.scalar.activation(
                warm[:], warm[:], mybir.ActivationFunctionType.Tanh
            )
            # All output blocks accumulate here; a single 4KB-descriptor DMA
            # writes them out at the end (host unpermutes rows).
            s_acc = cpool.tile([P, ND, LK], bf16, tag="sacc")

            # Projections: dst = W^T @ srcT accumulated over ND chunks of D.
            # One big DMA per input tensor -- a single InstDMACopy is split
            # across all 16 SDMA engines of its queue, so this parallelizes
            # better than per-chunk DMAs (which serialize on SWDGE issue).
            qT = cpool.tile([H, LQ], f32, tag="qT")
            kT = cpool.tile([H, LK], f32, tag="kT")
            srcs = {}
            # Weight tensors lead their queue (FIFO per queue): wk before ks
            # on sync, wq before qs on gpsimd -- each projection's operands
            # arrive in dependency order on its own queue.
            for name, dram, eng in (("wk", Wk, nc.sync), ("wq", Wq, nc.gpsimd)):
                t = lpool.tile([P, ND, H], bf16, tag=name)
                eng.dma_start(
                    t[:], dram[:, :].rearrange("(p c) k -> p c k", p=P)
                )
                srcs[name] = t
            # "(p c)" split: partition p holds DRAM rows 4p..4p+3, i.e. 4KB
            # contiguous per partition -> 4x bigger DMA descriptors. The
            # projection contracts over all of D either way, so the permuted
            # d-to-partition mapping is harmless (both operands use the same
            # mapping). Both ride the sync HWDGE queue: each InstDMACopy is
            # split across all 16 SDMA engines, and the scalar queue must be
            # left alone (it starves once ACT saturates).
            for name, dram, eng in (("ks", ksT, nc.sync), ("qs", qsT, nc.gpsimd)):
                t = lpool.tile([P, ND, LK], bf16, tag=name)
                src_r = dram[:, :].rearrange("(p c) k -> p c k", p=P)
                # Two halves so the first projection matmuls can start while
                # the second half streams; ks and qs ride different queues
                # (sync HWDGE / gpsimd SWDGE) so they stream in parallel.
                eng.dma_start(t[:, : ND // 2, :], src_r[:, : ND // 2, :])
                eng.dma_start(t[:, ND // 2 :, :], src_r[:, ND // 2 :, :])
                srcs[name] = t
            # wbig is not needed until the first reduce-matmul (~22us), so it
            # goes last on the gpsimd queue.
            wb = cpool.tile([P, 2 * H - 1], bf16, tag="wb")
            nc.gpsimd.dma_start(wb[:], wbig[:])
            for sname, wname, dst in (("ks", "wk", kT), ("qs", "wq", qT)):
                ps = ppool.tile([H, LQ], f32, tag="proj")
                for c in range(ND):
                    nc.tensor.matmul(
                        ps[:],
                        srcs[wname][:, c, :],
                        srcs[sname][:, c, :],
                        start=(c == 0),
                        stop=(c == ND - 1),
                    )
                nc.vector.tensor_copy(dst[:], ps[:])

            tanh = mybir.ActivationFunctionType.Tanh
            # Variable group sizes: small groups at the start (pipeline ramp:
            # ACT can begin after only 8 DVE adds) and a split last group so
            # the final reduce-matmuls overlap the tanh tail.
            sizes = [8] * 6 + [16] * 29
            assert sum(sizes) == LQ
            sp = None
            lq0 = 0
            for gidx, n in enumerate(sizes):
                if lq0 % P == 0:
                    sp = sppool.tile([P, LK], f32, tag="spsum")
                y = ypool.tile([P, n * LK], bf16, tag="y")
                x = xpool.tile([P, n * LK], f32, tag="x")
                for j in range(n):
                    lq = lq0 + j
                    nc.vector.tensor_scalar_add(
                        x[:, j * LK : (j + 1) * LK], kT[:], qT[:, lq : lq + 1]
                    )
                if gidx == 0:
                    h = n // 2 * LK
                    nc.scalar.activation(y[:, :h], x[:, :h], tanh)
                    nc.scalar.activation(y[:, h:], x[:, h:], tanh)
                elif gidx == len(sizes) - 1:
                    # 4-way split so the final reduce-matmuls overlap the
                    # tanh tail without extra group transitions.
                    q4 = n // 4 * LK
                    for s in range(4):
                        nc.scalar.activation(
                            y[:, s * q4 : (s + 1) * q4], x[:, s * q4 : (s + 1) * q4], tanh
                        )
                else:
                    nc.scalar.activation(y[:], x[:], tanh)
                for j in range(n):
                    r = (lq0 + j) % P
                    nc.tensor.matmul(
                        sp[:],
                        wb[:, H - 1 - r : 2 * H - 1 - r],
                        y[:, j * LK : (j + 1) * LK],
                        start=(r == 0),
                        stop=(r == P - 1),
                    )
                lq0 += n
                if lq0 % P == 0:
                    nc.vector.tensor_copy(s_acc[:, lq0 // P - 1, :], sp[:])
            # One DMA for the whole output: partition p holds S rows
            # {blk*128+p}, written to DRAM rows 4p+blk (4KB contiguous per
            # partition); the host transposes them back.
            nc.sync.dma_start(
                out[:, :].rearrange("(p c) k -> p c k", p=P), s_acc[:]
            )

    nc.compile()
    return nc


def _in_maps(qs, ks, Wq, Wk, wv):
    import ml_dtypes

    bf = ml_dtypes.bfloat16
    wbig = np.zeros((H, 2 * H - 1), np.float32)
    wbig[:, H - 1] = wv
    wbig = wbig.astype(bf)
    Wq_b = np.ascontiguousarray(Wq, dtype=np.float32).astype(bf)
    Wk_b = np.ascontiguousarray(Wk, dtype=np.float32).astype(bf)
    qs = np.asarray(qs)
    ks = np.asarray(ks)
    maps = []
    for b in range(B):
        maps.append(
            {
                "qsT": np.ascontiguousarray(qs[b].T).astype(bf),
                "ksT": np.ascontiguousarray(ks[b].T).astype(bf),
                "Wq": Wq_b,
                "Wk": Wk_b,
                "wbig": wbig,
            }
        )
    return maps


def _unpermute(o):
    # DRAM row p*4+blk holds S row blk*128+p (see the final out-DMA).
    o = np.asarray(o, dtype=np.float32)
    return o.reshape(P, ND, LK).transpose(1, 0, 2).reshape(LQ, LK)


def run(qs, ks, Wq, Wk, wv, trace=False):
    import time

    from concourse.bass_utils import run_bass_kernel_spmd

    if "nc" not in _cache:
        _cache["nc"] = _build()
    in_maps = _in_maps(qs, ks, Wq, Wk, wv)
    last_err = None
    for attempt in range(3):
        try:
            res = run_bass_kernel_spmd(
                _cache["nc"], in_maps, core_ids=list(range(B)), trace=trace
            )
            break
        except Exception as e:  # transient NRT_EXEC_UNIT_UNRECOVERABLE etc.
            last_err = e
            time.sleep(5 * (attempt + 1))
    else:
        raise last_err
    outs = np.stack([_unpermute(res.results[i]["out"]) for i in range(B)], axis=0)
    return outs, res


def kernel(qs, ks, Wq, Wk, wv):
    out, _ = run(qs, ks, Wq, Wk, wv, trace=False)
    return out


# revision 42
# speedup vs baseline: 1.0103x; 1.0053x over previous
"""Additive attention scorer: S[b,q,k] = sum_h wv[h] * tanh((qs@Wq)[b,q,h] + (ks@Wk)[b,k,h]).

Sharding: data-parallel over batch B=8 across the 8 NeuronCores (one batch
element per core). Per core:
  1. PE projects qT = Wq^T @ qs[b]^T and kT = Wk^T @ ks[b]^T  -> [H=128, 512]
     (H lands exactly on the 128 partitions). Inputs come in bf16 (the
     tanh-argument error budget tolerates it; halves DMA and keeps the
     matmuls single-pass -- fp32 matmuls decompose into 2 HW passes).
  2. For each query row lq: DVE builds X = kT + qT[:, lq] (per-partition
     scalar broadcast add, fp32 2x_2P mode), batched ~16 rows per tile.
  3. ACT applies one big tanh over the batched tile, fp32-in/bf16-out
     (amortizes the per-instruction overhead; fp32 reads run at ~1.02
     cyc/elem vs 1.25 for bf16 reads). ACT is the hard bottleneck:
     512*512*128 tanh / 128 lanes / 1.2GHz ~= 218us/core.
  4. PE reduces over H with a shifted-wv stationary trick: lhsT is a
     [128,128] window of a [128,255] bf16 tensor holding wv in column 127,
     so the single nonzero output row of each matmul lands on PSUM partition
     (lq mod 128); 128 matmuls accumulate one [128,512] fp32 output block.
  5. DVE copies PSUM->SBUF (bf16), one 4KB-descriptor DMA to DRAM at the end.
"""

import numpy as np

B, LQ, LK, D, H = 8, 512, 512, 512, 128
P = 128           # SBUF partitions
ND = D // P       # contraction chunks for the projections

_cache = {}


def _build():
    import concourse.bass as bass
    import concourse.tile as tile
    from concourse import bacc, mybir

    f32 = mybir.dt.float32
    bf16 = mybir.dt.bfloat16
    nc = bacc.Bacc("TRN2", target_bir_lowering=False, debug=False, num_devices=B)

    qsT = nc.dram_tensor("qsT", [D, LQ], bf16, kind="ExternalInput")
    ksT = nc.dram_tensor("ksT", [D, LK], bf16, kind="ExternalInput")
    Wq = nc.dram_tensor("Wq", [D, H], bf16, kind="ExternalInput")
    Wk = nc.dram_tensor("Wk", [D, H], bf16, kind="ExternalInput")
    wbig = nc.dram_tensor("wbig", [H, 2 * H - 1], bf16, kind="ExternalInput")
    out = nc.dram_tensor("out", [LQ, LK], bf16, kind="ExternalOutput")

    with tile.TileContext(nc) as tc:
        with (
            tc.tile_pool(name="const", bufs=1) as cpool,
            tc.tile_pool(name="load", bufs=1) as lpool,
            tc.tile_pool(name="x", bufs=3) as xpool,
            tc.tile_pool(name="y", bufs=3) as ypool,
            tc.tile_pool(name="ppsum", bufs=2, space=bass.MemorySpace.PSUM) as ppool,
            tc.tile_pool(name="spsum", bufs=2, space=bass.MemorySpace.PSUM) as sppool,
        ):
            # Pre-warm the ACT tanh table set while the input DMAs are in
            # flight (the PSEUDO_LOAD_ACT_FUNC_SET costs ~2.7us once).
            warm = cpool.tile([P, 1], f32, tag="warm")
            nc.vector.memset(warm[:], 0.0)
            nc.scalar.activation(
                warm[:], warm[:], mybir.ActivationFunctionType.Tanh
            )
            # All output blocks accumulate here; a single 4KB-descriptor DMA
            # writes them out at the end (host unpermutes rows).
            s_acc = cpool.tile([P, ND, LK], bf16, tag="sacc")

            # Projections: dst = W^T @ srcT accumulated over ND chunks of D.
            # One big DMA per input tensor -- a single InstDMACopy is split
            # across all 16 SDMA engines of its queue, so this parallelizes
            # better than per-chunk DMAs (which serialize on SWDGE issue).
            qT = cpool.tile([H, LQ], f32, tag="qT")
            kT = cpool.tile([H, LK], f32, tag="kT")
            srcs = {}
            # Weight tensors lead their queue (FIFO per queue): wk before ks
            # on sync, wq before qs on gpsimd -- each projection's operands
            # arrive in dependency order on its own queue.
            for name, dram, eng in (("wk", Wk, nc.sync), ("wq", Wq, nc.gpsimd)):
                t = lpool.tile([P, ND, H], bf16, tag=name)
                eng.dma_start(
                    t[:], dram[:, :].rearrange("(p c) k -> p c k", p=P)
                )
                srcs[name] = t
            # "(p c)" split: partition p holds DRAM rows 4p..4p+3, i.e. 4KB
            # contiguous per partition -> 4x bigger DMA descriptors. The
            # projection contracts over all of D either way, so the permuted
            # d-to-partition mapping is harmless (both operands use the same
            # mapping). Both ride the sync HWDGE queue: each InstDMACopy is
            # split across all 16 SDMA engines, and the scalar queue must be
            # left alone (it starves once ACT saturates).
            for name, dram, eng in (("ks", ksT, nc.sync), ("qs", qsT, nc.gpsimd)):
                t = lpool.tile([P, ND, LK], bf16, tag=name)
                src_r = dram[:, :].rearrange("(p c) k -> p c k", p=P)
                # Two halves so the first projection matmuls can start while
                # the second half streams; ks and qs ride different queues
                # (sync HWDGE / gpsimd SWDGE) so they stream in parallel.
                eng.dma_start(t[:, : ND // 2, :], src_r[:, : ND // 2, :])
                eng.dma_start(t[:, ND // 2 :, :], src_r[:, ND // 2 :, :])
                srcs[name] = t
            # wbig is not needed until the first reduce-matmul (~22us), so it
            # goes last on the gpsimd queue.
            wb = cpool.tile([P, 2 * H - 1], bf16, tag="wb")
            nc.gpsimd.dma_start(wb[:], wbig[:])
            for sname, wname, dst in (("ks", "wk", kT), ("qs", "wq", qT)):
                ps = ppool.tile([H, LQ], f32, tag="proj")
                for c in range(ND):
                    nc.tensor.matmul(
                        ps[:],
                        srcs[wname][:, c, :],
                        srcs[sname][:, c, :],
                        start=(c == 0),
                        stop=(c == ND - 1),
                    )
                if dst is qT:
                    # The first adds only read qT columns 0..47 (per-row
                    # scalars); a tiny staged copy ungates them ~0.6us
                    # before the full copy lands.
                    nc.vector.tensor_copy(dst[:, :48], ps[:, :48])
                    nc.vector.tensor_copy(dst[:, 48:], ps[:, 48:])
                else:
                    nc.vector.tensor_copy(dst[:], ps[:])

            tanh = mybir.ActivationFunctionType.Tanh
            # Variable group sizes: small groups at the start (pipeline ramp:
            # ACT can begin after only 8 DVE adds) and a split last group so
            # the final reduce-matmuls overlap the tanh tail.
            sizes = [8] * 6 + [16] * 29
            assert sum(sizes) == LQ
            sp = None
            lq0 = 0
            for gidx, n in enumerate(sizes):
                if lq0 % P == 0:
                    sp = sppool.tile([P, LK], f32, tag="spsum")
                y = ypool.tile([P, n * LK], bf16, tag="y")
                x = xpool.tile([P, n * LK], f32, tag="x")
                for j in range(n):
                    lq = lq0 + j
                    nc.vector.tensor_scalar_add(
                        x[:, j * LK : (j + 1) * LK], kT[:], qT[:, lq : lq + 1]
                    )
                if gidx == 0:
                    h = n // 2 * LK
                    nc.scalar.activation(y[:, :h], x[:, :h], tanh)
                    nc.scalar.activation(y[:, h:], x[:, h:], tanh)
                elif gidx == len(sizes) - 1:
                    # 4-way split so the final reduce-matmuls overlap the
                    # tanh tail without extra group transitions.
                    q4 = n // 4 * LK
                    for s in range(4):
                        nc.scalar.activation(
                            y[:, s * q4 : (s + 1) * q4], x[:, s * q4 : (s + 1) * q4], tanh
                        )
                else:
                    nc.scalar.activation(y[:], x[:], tanh)
                for j in range(n):
                    r = (lq0 + j) % P
                    nc.tensor.matmul(
                        sp[:],
                        wb[:, H - 1 - r : 2 * H - 1 - r],
                        y[:, j * LK : (j + 1) * LK],
                        start=(r == 0),
                        stop=(r == P - 1),
                    )
                lq0 += n
                if lq0 % P == 0:
                    nc.vector.tensor_copy(s_acc[:, lq0 // P - 1, :], sp[:])
            # One DMA for the whole output: partition p holds S rows
            # {blk*128+p}, written to DRAM rows 4p+blk (4KB contiguous per
            # partition); the host transposes them back.
            nc.sync.dma_start(
                out[:, :].rearrange("(p c) k -> p c k", p=P), s_acc[:]
            )

    nc.compile()
    return nc


def _in_maps(qs, ks, Wq, Wk, wv):
    import ml_dtypes

    bf = ml_dtypes.bfloat16
    wbig = np.zeros((H, 2 * H - 1), np.float32)
    wbig[:, H - 1] = wv
    wbig = wbig.astype(bf)
    Wq_b = np.ascontiguousarray(Wq, dtype=np.float32).astype(bf)
    Wk_b = np.ascontiguousarray(Wk, dtype=np.float32).astype(bf)
    qs = np.asarray(qs)
    ks = np.asarray(ks)
    maps = []
    for b in range(B):
        maps.append(
            {
                "qsT": np.ascontiguousarray(qs[b].T).astype(bf),
                "ksT": np.ascontiguousarray(ks[b].T).astype(bf),
                "Wq": Wq_b,
                "Wk": Wk_b,
                "wbig": wbig,
            }
        )
    return maps


def _unpermute(o):
    # DRAM row p*4+blk holds S row blk*128+p (see the final out-DMA).
    o = np.asarray(o, dtype=np.float32)
    return o.reshape(P, ND, LK).transpose(1, 0, 2).reshape(LQ, LK)


def run(qs, ks, Wq, Wk, wv, trace=False):
    import time

    from concourse.bass_utils import run_bass_kernel_spmd

    if "nc" not in _cache:
        _cache["nc"] = _build()
    in_maps = _in_maps(qs, ks, Wq, Wk, wv)
    last_err = None
    for attempt in range(3):
        try:
            res = run_bass_kernel_spmd(
                _cache["nc"], in_maps, core_ids=list(range(B)), trace=trace
            )
            break
        except Exception as e:  # transient NRT_EXEC_UNIT_UNRECOVERABLE etc.
            last_err = e
            time.sleep(5 * (attempt + 1))
    else:
        raise last_err
    outs = np.stack([_unpermute(res.results[i]["out"]) for i in range(B)], axis=0)
    return outs, res


def kernel(qs, ks, Wq, Wk, wv):
    out, _ = run(qs, ks, Wq, Wk, wv, trace=False)
    return out
